# revision 50
# baseline (speedup 1.0000x reference)
"""Trainium2 Bass kernel for nn_MultiHeadAttention (B=4, S=2048, C=256, H=8).

Sharding: data-parallel over (batch, seq) - 8 cores, core i handles
batch b = i//2 and query rows r0 = (i%2)*1024 .. r0+1024.  No collectives;
host concatenates the 8 row-shards.

Algebraic folding (host side, fp32):
  scores = (x Wq + bq)(x Wk + bk)^T -> x A x^T + u.x_t  with A = Wq Wk^T,
  u = Wk bq (the bk term is constant per query row, softmax-invariant).
  attn (x Wv + bv) Wfc = (attn x) M + bv Wfc  with M = Wv Wfc.
  So the device only computes: q' = x A + u (one proj per head), scores
  against x^T directly, attn-times-x, then fc with M.  K and V projections
  and their SBUF copies are gone.

Precision: fp8e4 (DoubleRow, 2x contraction per pass) for q' proj, scores,
rowsum and attn*x; bf16 for the small fc; fp32 PSUM accumulation, softmax
normalization and LayerNorm in fp32.  A is scaled by 16 on host so fp8
quantization of q' (sigma~16) stays in the normal range; the activation
scale folds the 1/16 back.  exp is shifted by -ln(16) (softmax-invariant)
so e values stay well under the fp8e4 max of 240.

LayerNorm rstd = exp(-0.5*ln(var+eps)) keeps the whole kernel on one
activation table set (natural_log_exp_and_others) - no table switches.
"""

import sys

for _p in ("/opt/trn_rl_repo",):
    if _p not in sys.path:
        sys.path.insert(0, _p)

from contextlib import ExitStack

import numpy as np

import concourse.bass as bass
from concourse import bacc
import concourse.tile as tile
from concourse import mybir

P = 128
B, S, C, H = 4, 2048, 256, 8
RQ = 1024            # query rows per core
CH = 512             # query-row chunk (matmul N)
NCH = RQ // CH       # chunks per core = 2
NT = S // P          # key tiles = 16
ND = C // P          # feature tiles = 2
NR = RQ // P         # row tiles per core = 8
EPS = 1e-5
SCALE = 1.0 / np.sqrt(C)          # 1/16
ESCALE = float(SCALE / 16.0)      # activation scale: q' carries an extra 16x
LN16 = float(np.log(16.0))

F32 = mybir.dt.float32
I32 = mybir.dt.int32
BF16 = mybir.dt.bfloat16
F8 = mybir.dt.float8e4
AF = mybir.ActivationFunctionType
OP = mybir.AluOpType
DR = mybir.MatmulPerfMode.DoubleRow


def build_nc() -> bass.Bass:
    nc = bacc.Bacc(None)

    xbt8 = nc.declare_dram_parameter("xbt8", [P, ND, S], F8, isOutput=False)
    xb8 = nc.declare_dram_parameter("xb8", [P, NT, C], F8, isOutput=False)
    xqf = nc.declare_dram_parameter("xqf", [P, NR, C], F32, isOutput=False)
    a8 = nc.declare_dram_parameter("a8", [P, ND, H, C], F8, isOutput=False)
    m8 = nc.declare_dram_parameter("m8", [P, ND, H, C], F8, isOutput=False)
    ub = nc.declare_dram_parameter("ub", [P, ND, H], F32, isOutput=False)
    # brow = concat(bfc_eff [256], gamma [256], beta [256])
    brow = nc.declare_dram_parameter("brow", [3 * C], F32, isOutput=False)
    out = nc.declare_dram_parameter("out", [RQ, C], F32, isOutput=True)

    with tile.TileContext(nc) as tc, ExitStack() as ctx:
        singles = ctx.enter_context(tc.tile_pool(name="singles", bufs=1))
        qpool = ctx.enter_context(tc.tile_pool(name="qpool", bufs=2))
        epool = ctx.enter_context(tc.tile_pool(name="epool", bufs=2))
        otpool = ctx.enter_context(tc.tile_pool(name="otpool", bufs=2))
        lnpool = ctx.enter_context(tc.tile_pool(name="lnpool", bufs=4))

        ps_sc = ctx.enter_context(tc.tile_pool(name="ps_sc", bufs=2, space="PSUM"))
        ps_rs = ctx.enter_context(tc.tile_pool(name="ps_rs", bufs=1, space="PSUM"))
        ps_ao = ctx.enter_context(tc.tile_pool(name="ps_ao", bufs=2, space="PSUM"))
        ps_sm = ctx.enter_context(tc.tile_pool(name="ps_sm", bufs=1, space="PSUM"))

        # ---- constants ----
        # rowsum weights 1/32 so ot = 32*ao/rowsum stays in fp8e4 range
        # (|attn-weighted x| <= ~6, 32*6 = 192 < 240); fc de-scales by 1/2048.
        ones8 = singles.tile([P, ND, P], F8)
        nc.vector.memset(ones8, 1.0 / 32.0)
        expb = singles.tile([P, 1], F32)
        nc.vector.memset(expb, -LN16)
        # preload the exp table set while input DMAs are in flight
        tl_t = singles.tile([P, 1], F32)
        nc.scalar.activation(out=tl_t, in_=expb, func=AF.Exp, bias=expb,
                             scale=1.0)

        # ---- input DMAs (all into persistent tiles).  The scalar (ACT)
        # queue gets ONLY the first a8 piece: DMA instructions on that queue
        # would otherwise delay the first exp by several us. ----
        ub_sb = singles.tile([P, ND, H], F32, tag="ub", name="ub_sb")
        nc.sync.dma_start(out=ub_sb, in_=ub[:])
        xbt_sb = singles.tile([P, ND, S], F8, tag="xbt", name="xbt_sb")
        for q4 in range(4):
            eng = nc.gpsimd if q4 % 2 == 0 else nc.sync
            eng.dma_start(out=xbt_sb[:, :, q4 * CH:(q4 + 1) * CH],
                          in_=xbt8[:, :, q4 * CH:(q4 + 1) * CH])
        a8_sb = singles.tile([P, ND, H, C], F8, tag="a8", name="a8_sb")
        nc.scalar.dma_start(out=a8_sb[:, :, 0:2], in_=a8[:, :, 0:2])
        xb8_sb = singles.tile([P, NT, C], F8, tag="xb8", name="xb8_sb")
        for q8_ in range(0, NT, 8):
            eng = nc.gpsimd if q8_ == 0 else nc.sync
            eng.dma_start(out=xb8_sb[:, q8_:q8_ + 8],
                          in_=xb8[:, q8_:q8_ + 8])
        for hh in range(2, H, 2):
            eng = nc.gpsimd if hh < 6 else nc.sync
            eng.dma_start(out=a8_sb[:, :, hh:hh + 2],
                          in_=a8[:, :, hh:hh + 2])
        m8_sb = singles.tile([P, ND, H, C], F8, tag="m8", name="m8_sb")
        for hh in range(0, H, 4):
            nc.sync.dma_start(out=m8_sb[:, :, hh:hh + 4],
                              in_=m8[:, :, hh:hh + 4])
        brow_sb = singles.tile([P, 3 * C], F32, tag="brow", name="brow_sb")
        brow_ap = brow[:]
        brow_bc = bass.AP(tensor=brow_ap.tensor, offset=brow_ap.offset,
                          ap=[[0, P]] + list(brow_ap.ap))
        nc.gpsimd.dma_start(out=brow_sb, in_=brow_bc)
        bfc_sb = brow_sb[:, 0:C]
        # gamma/beta replicated 4x so the LN epilogue applies them in one
        # FD=1024 op over 4 row-tiles
        gb4_sb = singles.tile([P, 2, 4, C], F32, tag="gb4", name="gb4_sb")
        gamma4_sb = gb4_sb[:, 0]
        beta4_sb = gb4_sb[:, 1]

        def fill_gb4():
            # deferred: DVE-FIFO ordering — must not precede the qproj casts
            for gi in range(2):
                for rep in range(4):
                    nc.vector.tensor_copy(
                        out=gb4_sb[:, gi, rep],
                        in_=brow_sb[:, (1 + gi) * C:(2 + gi) * C])
        xr_sb = singles.tile([P, NR, C], F32, tag="xr", name="xr_sb")
        for q8_ in range(0, NR, 4):
            eng = nc.gpsimd if q8_ == 0 else nc.sync
            eng.dma_start(out=xr_sb[:, q8_:q8_ + 4],
                          in_=xqf[:, q8_:q8_ + 4])

        # ---- fc accumulator (fp32, SBUF) ----
        acc_sb = singles.tile([P, NR, C], F32, tag="acc", name="acc_sb")

        # ---- warmup: get the HAM clock gate to 2.4 GHz while DMAs land.
        # One PSUM accumulation group -> back-to-back MMs, no inter-MM sems.
        def warm(n, pool=None, tag=None):
            wps = (pool or ps_rs).tile([P, P], F32, tag=tag or "rs",
                                       name="wps")
            for i in range(n):
                nc.tensor.matmul(wps, lhsT=ones8, rhs=ones8,
                                 start=(i == 0), stop=(i == n - 1),
                                 perf_mode=DR)

        warm(12)

        # ---- q' projection: q'^T[co, r] = A^T x^T + u (fp8 out, 16x scale) ----
        q_tiles = {}

        def make_qproj_steps(h, pools=None):
            qt = qpool.tile([P, ND, RQ], F8, tag="q8", name=f"q8_{h}")
            q_tiles[h] = qt

            def step(r2, co2, pool, tag):
                def go():
                    qps = pool.tile([P, CH], F32, tag=tag, name="qps")
                    nc.tensor.matmul(
                        qps,
                        lhsT=a8_sb[:, :, h, co2 * P:(co2 + 1) * P],
                        rhs=xbt_sb[:, :, r2 * CH:(r2 + 1) * CH],
                        start=True, stop=True, perf_mode=DR,
                    )
                    nc.vector.tensor_scalar_add(
                        out=qt[:, co2, r2 * CH:(r2 + 1) * CH], in0=qps,
                        scalar1=ub_sb[:, co2, h:h + 1],
                    )
                return go

            if pools is None:
                pools = [(ps_sm, "sm")] * 4
            return [step(r2, co2, *pools[r2 * ND + co2])
                    for r2 in range(NCH) for co2 in range(ND)]

        # h=0 projection up front: 4 MMs land in 4 different PSUM banks so
        # they stream back-to-back as soon as the a8/xbt DMA pieces arrive
        warm(28)
        for st in make_qproj_steps(0, pools=[(ps_sm, "sm"), (ps_rs, "rs"),
                                             (ps_ao, "ao"), (ps_ao, "ao")]):
            st()
        warm(12, pool=ps_sc, tag="sc")

        # ---- init acc = x + bfc_eff (residual folded in before head 0) ----
        def init_acc(i):
            nc.vector.scalar_tensor_tensor(
                out=acc_sb[:, i], in0=xr_sb[:, i], scalar=1.0, in1=bfc_sb,
                op0=OP.mult, op1=OP.add)

        # ---- LayerNorm: per-row stats, then a 4-row batched rsqrt chain ----
        out_r = out.rearrange("(n p) d -> p n d", p=P)
        ln_mv = {}

        def emit_ln_stats(i):
            stats = lnpool.tile([P, 6], F32, tag="stats")
            nc.vector.bn_stats(out=stats, in_=acc_sb[:, i])
            mv = lnpool.tile([P, 2], F32, tag="mv", name=f"mv{i}")
            nc.vector.bn_aggr(out=mv, in_=stats)
            ln_mv[i] = mv

        def emit_ln_finish(idxs):
            # rstd = 1/sqrt(var+eps) for all rows at once, DVE-only
            # (quake seed + 2 Newton steps) - no ACT table switch.
            n = len(idxs)
            ve = lnpool.tile([P, n], F32, tag="ve")
            for k, i in enumerate(idxs):
                nc.vector.tensor_scalar_add(out=ve[:, k:k + 1],
                                            in0=ln_mv[i][:, 1:2], scalar1=EPS)
            y = lnpool.tile([P, n], F32, tag="y")
            tn = lnpool.tile([P, n], F32, tag="tn")
            nc.vector.tensor_scalar(out=y.bitcast(I32), in0=ve.bitcast(I32),
                                    scalar1=1, scalar2=-1,
                                    op0=OP.arith_shift_right,
                                    op1=OP.bitwise_xor)
            nc.vector.tensor_scalar(out=y.bitcast(I32), in0=y.bitcast(I32),
                                    scalar1=0x5f3759df + 1, scalar2=None,
                                    op0=OP.add)
            # one Newton step: max rel err ~0.17%, well inside tolerance
            nc.vector.tensor_tensor(out=tn, in0=y, in1=y, op=OP.mult)
            nc.vector.tensor_tensor(out=tn, in0=tn, in1=ve, op=OP.mult)
            nc.vector.tensor_scalar(out=tn, in0=tn, scalar1=-0.5,
                                    scalar2=1.5, op0=OP.mult, op1=OP.add)
            nc.vector.tensor_tensor(out=y, in0=y, in1=tn, op=OP.mult)
            for k, i in enumerate(idxs):
                t = acc_sb[:, i]
                nc.vector.tensor_scalar(out=t, in0=t, scalar1=ln_mv[i][:, 0:1],
                                        scalar2=y[:, k:k + 1],
                                        op0=OP.subtract, op1=OP.mult)
            i0, i1 = min(idxs), max(idxs) + 1
            blk = acc_sb[:, i0:i1]
            nc.vector.tensor_tensor(out=blk, in0=blk, in1=gamma4_sb,
                                    op=OP.mult)
            nc.vector.tensor_tensor(out=blk, in0=blk, in1=beta4_sb, op=OP.add)
            nc.gpsimd.dma_start(out=out_r[:, i0:i1, :], in_=acc_sb[:, i0:i1])

        def emit_fc(ot_sb, fh, fch, final, r1s=None, last_part=True):
            # the final chunk's fc spreads over the freed rs/ao banks so the
            # 4 matmuls stream without waiting on per-bank DVE evacuation
            fin_pools = [(ps_sm, "sm"), (ps_rs, "rs"),
                         (ps_ao, "ao"), (ps_ao, "ao")]
            for r1 in (r1s if r1s is not None else range(CH // P)):
                idx = fch * (CH // P) + r1
                pool, ptag = fin_pools[r1] if (final and fch == NCH - 1) \
                    else (ps_sm, "sm")
                fcp = pool.tile([P, C], F32, tag=ptag, name="fcp")
                nc.tensor.matmul(
                    fcp,
                    lhsT=ot_sb[:, :, r1 * P:(r1 + 1) * P],
                    rhs=m8_sb[:, :, fh, :],
                    start=True, stop=True, perf_mode=DR,
                )
                # acc += fcp/2048 (ot carries 32x, M carries 64x)
                nc.vector.scalar_tensor_tensor(
                    out=acc_sb[:, idx], in0=fcp, scalar=1.0 / 2048.0,
                    in1=acc_sb[:, idx], op0=OP.mult, op1=OP.add)
                if final:
                    emit_ln_stats(idx)
            if final and last_part:
                emit_ln_finish([fch * (CH // P) + r1 for r1 in range(CH // P)])

        for i in range(NR):
            init_acc(i)
        fill_gb4()

        # ---- head loop, software-pipelined across chunk boundaries: each
        # chunk's last two rs/ao groups, softmax normalize, and fc are
        # deferred into the NEXT chunk's early iterations so neither the PE
        # nor the ACT ever drains at a boundary. ----
        def make_chunk_state(h, ch):
            qt = q_tiles[h]
            rsl = slice(ch * CH, (ch + 1) * CH)
            st = {
                "h": h, "ch": ch, "qt": qt, "rsl": rsl,
                "e8": epool.tile([P, NT, CH], F8, tag="e", name=f"e{h}{ch}"),
                "rs": None, "ao": None, "ot": None, "rcp": None,
            }
            return st

        def emit_rs(st, j):
            if st["rs"] is None:
                st["rs"] = ps_rs.tile([P, CH], F32, tag="rs", name="rs")
            nc.tensor.matmul(st["rs"], lhsT=ones8,
                             rhs=st["e8"][:, 2 * j:2 * j + 2, :],
                             start=(j == 0), stop=(j == NT // 2 - 1),
                             perf_mode=DR)
            if j == NT // 2 - 1:
                rcp = otpool.tile([P, CH], F32, tag="rcp")
                nc.vector.reciprocal_approx_fast(out=rcp, in_=st["rs"])
                st["rcp"] = rcp

        def emit_ao(st, j):
            if st["ao"] is None:
                st["ao"] = [ps_ao.tile([P, CH], F32, tag="ao", name=f"ao{c2}")
                            for c2 in range(ND)]
            for c2 in range(ND):
                nc.tensor.matmul(
                    st["ao"][c2],
                    lhsT=xb8_sb[:, 2 * j:2 * j + 2, c2 * P:(c2 + 1) * P],
                    rhs=st["e8"][:, 2 * j:2 * j + 2, :],
                    start=(j == 0), stop=(j == NT // 2 - 1),
                    perf_mode=DR,
                )
            if j == NT // 2 - 1:
                ot_sb = otpool.tile([P, ND, CH], F8, tag="ot")
                for c2 in range(ND):
                    nc.vector.tensor_tensor(out=ot_sb[:, c2], in0=st["ao"][c2],
                                            in1=st["rcp"][:], op=OP.mult)
                st["ot"] = ot_sb

        def emit_norm(st):
            rcp = otpool.tile([P, CH], F32, tag="rcp")
            nc.vector.reciprocal_approx_fast(out=rcp, in_=st["rs"])
            ot_sb = otpool.tile([P, ND, CH], F8, tag="ot")
            for c2 in range(ND):
                nc.vector.tensor_tensor(
                    out=ot_sb[:, c2], in0=st["ao"][c2], in1=rcp[:], op=OP.mult)
            st["ot"] = ot_sb

        # Uniform per-j schedule: every iteration carries sc-pair + one rs
        # group (3-deep lag) + one ao group (4-deep lag), so PE load per j
        # is nearly constant and the ACT exp stream never starves.
        chunks = [(h, ch) for h in range(H) for ch in range(NCH)]
        prev = None
        qsteps = []
        NH = NT // 2
        for h, ch in chunks:
            if ch == 0:
                qsteps = make_qproj_steps(h + 1) if h + 1 < H else []
            cur = make_chunk_state(h, ch)
            for j in range(NH):
                scp = ps_sc.tile([P, 2, CH], F32, tag="sc", name="scp")
                for tt in range(2):
                    t = 2 * j + tt
                    nc.tensor.matmul(
                        scp[:, tt],
                        lhsT=xbt_sb[:, :, t * P:(t + 1) * P],
                        rhs=cur["qt"][:, :, cur["rsl"]],
                        start=True, stop=True, perf_mode=DR,
                    )
                # e = exp(scores*SCALE - ln16), fp8; FD=1024 per op
                nc.scalar.activation(out=cur["e8"][:, 2 * j:2 * j + 2],
                                     in_=scp, func=AF.Exp, bias=expb,
                                     scale=ESCALE)
                if prev is not None:
                    if j <= 2:
                        emit_rs(prev, NH - 3 + j)      # rs(5),(6),(7)+rcp
                    if j <= 3:
                        emit_ao(prev, NH - 4 + j)      # ao(4)..(7)+ot
                if j >= 3:
                    emit_rs(cur, j - 3)
                if j >= 4:
                    emit_ao(cur, j - 4)
                if prev is not None:
                    fin = prev["h"] == H - 1
                    if j == 5:
                        emit_fc(prev["ot"], prev["h"], prev["ch"], fin,
                                r1s=(0, 1), last_part=False)
                    elif j == 6:
                        emit_fc(prev["ot"], prev["h"], prev["ch"], fin,
                                r1s=(2, 3), last_part=True)
                        prev = None
                if j in (3, 4) and qsteps:
                    qsteps.pop(0)()
            prev = cur
        # flush the final chunk
        for j in range(NH - 3, NH):
            emit_rs(prev, j)
        for j in range(NH - 4, NH):
            emit_ao(prev, j)
        emit_fc(prev["ot"], prev["h"], prev["ch"], True)

    nc.finalize()
    return nc


_NC = None


def _get_nc():
    global _NC
    if _NC is None:
        _NC = build_nc()
    return _NC


def make_in_maps(inputs):
    import ml_dtypes
    f8 = ml_dtypes.float8_e4m3

    x = np.asarray(inputs["x"], dtype=np.float32)
    Wq = np.asarray(inputs["Wq"], np.float32)
    Wk = np.asarray(inputs["Wk"], np.float32)
    Wv = np.asarray(inputs["Wv"], np.float32)
    Wfc = np.asarray(inputs["Wfc"], np.float32)
    bq = np.asarray(inputs["bq"], np.float32)
    bv = np.asarray(inputs["bv"], np.float32)
    bfc = np.asarray(inputs["bfc"], np.float32)
    gamma = np.asarray(inputs["gamma"], np.float32)
    beta = np.asarray(inputs["beta"], np.float32)

    # host-side folds (fp32)
    A = Wq @ Wk.transpose(0, 2, 1)                   # [H, C, C]
    u = np.einsum('hcd,hd->hc', Wk, bq)              # [H, C]
    M = Wv @ Wfc.reshape(H, C, C)                    # [H, C, C]
    bfc_eff = bfc + bv.ravel() @ Wfc

    a8_np = np.clip(16.0 * A, -240, 240).astype(f8)
    # [H, C, C] -> [P, ND, H, C]: (p, j, h, co) = A[h, j*128+p, co]
    a8_np = np.ascontiguousarray(
        a8_np.reshape(H, ND, P, C).transpose(2, 1, 0, 3))
    m8_np = np.clip(64.0 * M, -240, 240).astype(f8)
    m8_np = np.ascontiguousarray(
        m8_np.reshape(H, ND, P, C).transpose(2, 1, 0, 3))
    ub_np = np.ascontiguousarray((16.0 * u).reshape(H, ND, P).transpose(2, 1, 0))
    brow_np = np.ascontiguousarray(
        np.concatenate([bfc_eff.ravel(), gamma.ravel(), beta.ravel()]))

    shared = {"a8": a8_np, "m8": m8_np, "ub": ub_np, "brow": brow_np}
    in_maps = []
    for core in range(8):
        b, r0 = core // 2, (core % 2) * RQ
        x8r = np.roll(x[b].astype(f8), -r0, axis=0)          # [S, C] fp8
        m = dict(shared)
        # x^T: (p, j, t) = x8r[t, j*128+p]
        m["xbt8"] = np.ascontiguousarray(
            x8r.T.reshape(ND, P, S).transpose(1, 0, 2))
        # x rows: (p, n, c) = x8r[n*128+p, c]
        m["xb8"] = np.ascontiguousarray(
            x8r.reshape(NT, P, C).transpose(1, 0, 2))
        m["xqf"] = np.ascontiguousarray(
            x[b, r0:r0 + RQ].reshape(NR, P, C).transpose(1, 0, 2))
        in_maps.append(m)
    return in_maps


def assemble(results):
    out = np.empty((B, S, C), dtype=np.float32)
    for core in range(8):
        b, r0 = core // 2, (core % 2) * RQ
        out[b, r0:r0 + RQ] = results[core]["out"]
    return out


def kernel(**inputs) -> np.ndarray:
    from concourse.bass_utils import run_bass_kernel_spmd

    nc = _get_nc()
    in_maps = make_in_maps(inputs)
    res = run_bass_kernel_spmd(nc, in_maps, core_ids=list(range(8)))
    return assemble(res.results)


# revision 52
# speedup vs baseline: 1.0189x; 1.0189x over previous
"""Trainium2 Bass kernel for nn_MultiHeadAttention (B=4, S=2048, C=256, H=8).

Sharding: data-parallel over (batch, seq) - 8 cores, core i handles
batch b = i//2 and query rows r0 = (i%2)*1024 .. r0+1024.  No collectives;
host concatenates the 8 row-shards.

Algebraic folding (host side, fp32):
  scores = (x Wq + bq)(x Wk + bk)^T -> x A x^T + u.x_t  with A = Wq Wk^T,
  u = Wk bq (the bk term is constant per query row, softmax-invariant).
  attn (x Wv + bv) Wfc = (attn x) M + bv Wfc  with M = Wv Wfc.
  So the device only computes: q' = x A + u (one proj per head), scores
  against x^T directly, attn-times-x, then fc with M.  K and V projections
  and their SBUF copies are gone.

Precision: fp8e4 (DoubleRow, 2x contraction per pass) for q' proj, scores,
rowsum and attn*x; bf16 for the small fc; fp32 PSUM accumulation, softmax
normalization and LayerNorm in fp32.  A is scaled by 16 on host so fp8
quantization of q' (sigma~16) stays in the normal range; the activation
scale folds the 1/16 back.  exp is shifted by -ln(16) (softmax-invariant)
so e values stay well under the fp8e4 max of 240.

LayerNorm rstd = exp(-0.5*ln(var+eps)) keeps the whole kernel on one
activation table set (natural_log_exp_and_others) - no table switches.
"""

import sys

for _p in ("/opt/trn_rl_repo",):
    if _p not in sys.path:
        sys.path.insert(0, _p)

from contextlib import ExitStack

import numpy as np

import concourse.bass as bass
from concourse import bacc
import concourse.tile as tile
from concourse import mybir

P = 128
B, S, C, H = 4, 2048, 256, 8
RQ = 1024            # query rows per core
CH = 512             # query-row chunk (matmul N)
NCH = RQ // CH       # chunks per core = 2
NT = S // P          # key tiles = 16
ND = C // P          # feature tiles = 2
NR = RQ // P         # row tiles per core = 8
EPS = 1e-5
SCALE = 1.0 / np.sqrt(C)          # 1/16
ESCALE = float(SCALE / 16.0)      # activation scale: q' carries an extra 16x
LN16 = float(np.log(16.0))

F32 = mybir.dt.float32
I32 = mybir.dt.int32
BF16 = mybir.dt.bfloat16
F8 = mybir.dt.float8e4
AF = mybir.ActivationFunctionType
OP = mybir.AluOpType
DR = mybir.MatmulPerfMode.DoubleRow


def build_nc() -> bass.Bass:
    nc = bacc.Bacc(None)

    xbt8 = nc.declare_dram_parameter("xbt8", [P, ND, S], F8, isOutput=False)
    xb8 = nc.declare_dram_parameter("xb8", [P, NT, C], F8, isOutput=False)
    xqf = nc.declare_dram_parameter("xqf", [P, NR, C], F32, isOutput=False)
    a8 = nc.declare_dram_parameter("a8", [P, ND, H, C], F8, isOutput=False)
    m8 = nc.declare_dram_parameter("m8", [P, ND, H, C], F8, isOutput=False)
    ub = nc.declare_dram_parameter("ub", [P, ND, H], F32, isOutput=False)
    # brow = concat(bfc_eff [256], gamma [256], beta [256])
    brow = nc.declare_dram_parameter("brow", [3 * C], F32, isOutput=False)
    out = nc.declare_dram_parameter("out", [RQ, C], F32, isOutput=True)

    with tile.TileContext(nc) as tc, ExitStack() as ctx:
        singles = ctx.enter_context(tc.tile_pool(name="singles", bufs=1))
        qpool = ctx.enter_context(tc.tile_pool(name="qpool", bufs=2))
        epool = ctx.enter_context(tc.tile_pool(name="epool", bufs=2))
        otpool = ctx.enter_context(tc.tile_pool(name="otpool", bufs=2))
        lnpool = ctx.enter_context(tc.tile_pool(name="lnpool", bufs=4))

        ps_sc = ctx.enter_context(tc.tile_pool(name="ps_sc", bufs=2, space="PSUM"))
        ps_rs = ctx.enter_context(tc.tile_pool(name="ps_rs", bufs=1, space="PSUM"))
        ps_ao = ctx.enter_context(tc.tile_pool(name="ps_ao", bufs=2, space="PSUM"))
        ps_sm = ctx.enter_context(tc.tile_pool(name="ps_sm", bufs=1, space="PSUM"))

        # ---- constants ----
        # rowsum weights 1/32 so ot = 32*ao/rowsum stays in fp8e4 range
        # (|attn-weighted x| <= ~6, 32*6 = 192 < 240); fc de-scales by 1/2048.
        ones8 = singles.tile([P, ND, P], F8)
        nc.vector.memset(ones8, 1.0 / 32.0)
        expb = singles.tile([P, 1], F32)
        nc.vector.memset(expb, -LN16)
        # preload the exp table set while input DMAs are in flight
        tl_t = singles.tile([P, 1], F32)
        nc.scalar.activation(out=tl_t, in_=expb, func=AF.Exp, bias=expb,
                             scale=1.0)

        # ---- input DMAs (all into persistent tiles).  The scalar (ACT)
        # queue gets ONLY the first a8 piece: DMA instructions on that queue
        # would otherwise delay the first exp by several us. ----
        ub_sb = singles.tile([P, ND, H], F32, tag="ub", name="ub_sb")
        nc.sync.dma_start(out=ub_sb, in_=ub[:])
        xbt_sb = singles.tile([P, ND, S], F8, tag="xbt", name="xbt_sb")
        for q4 in range(4):
            eng = nc.gpsimd if q4 % 2 == 0 else nc.sync
            eng.dma_start(out=xbt_sb[:, :, q4 * CH:(q4 + 1) * CH],
                          in_=xbt8[:, :, q4 * CH:(q4 + 1) * CH])
        a8_sb = singles.tile([P, ND, H, C], F8, tag="a8", name="a8_sb")
        nc.scalar.dma_start(out=a8_sb[:, :, 0:2], in_=a8[:, :, 0:2])
        xb8_sb = singles.tile([P, NT, C], F8, tag="xb8", name="xb8_sb")
        for q8_ in range(0, NT, 8):
            eng = nc.gpsimd if q8_ == 0 else nc.sync
            eng.dma_start(out=xb8_sb[:, q8_:q8_ + 8],
                          in_=xb8[:, q8_:q8_ + 8])
        for hh in range(2, H, 2):
            eng = nc.gpsimd if hh < 6 else nc.sync
            eng.dma_start(out=a8_sb[:, :, hh:hh + 2],
                          in_=a8[:, :, hh:hh + 2])
        m8_sb = singles.tile([P, ND, H, C], F8, tag="m8", name="m8_sb")
        for hh in range(0, H, 4):
            nc.sync.dma_start(out=m8_sb[:, :, hh:hh + 4],
                              in_=m8[:, :, hh:hh + 4])
        brow_sb = singles.tile([P, 3 * C], F32, tag="brow", name="brow_sb")
        brow_ap = brow[:]
        brow_bc = bass.AP(tensor=brow_ap.tensor, offset=brow_ap.offset,
                          ap=[[0, P]] + list(brow_ap.ap))
        nc.gpsimd.dma_start(out=brow_sb, in_=brow_bc)
        bfc_sb = brow_sb[:, 0:C]
        # gamma/beta replicated 4x so the LN epilogue applies them in one
        # FD=1024 op over 4 row-tiles
        gb4_sb = singles.tile([P, 2, 4, C], F32, tag="gb4", name="gb4_sb")
        gamma4_sb = gb4_sb[:, 0]
        beta4_sb = gb4_sb[:, 1]

        def fill_gb4():
            # deferred: DVE-FIFO ordering — must not precede the qproj casts
            for gi in range(2):
                for rep in range(4):
                    nc.vector.tensor_copy(
                        out=gb4_sb[:, gi, rep],
                        in_=brow_sb[:, (1 + gi) * C:(2 + gi) * C])
        xr_sb = singles.tile([P, NR, C], F32, tag="xr", name="xr_sb")
        for q8_ in range(0, NR, 4):
            eng = nc.gpsimd if q8_ == 0 else nc.sync
            eng.dma_start(out=xr_sb[:, q8_:q8_ + 4],
                          in_=xqf[:, q8_:q8_ + 4])

        # ---- fc accumulator (fp32, SBUF) ----
        acc_sb = singles.tile([P, NR, C], F32, tag="acc", name="acc_sb")

        # ---- warmup: get the HAM clock gate to 2.4 GHz while DMAs land.
        # One PSUM accumulation group -> back-to-back MMs, no inter-MM sems.
        def warm(n, pool=None, tag=None):
            wps = (pool or ps_rs).tile([P, P], F32, tag=tag or "rs",
                                       name="wps")
            for i in range(n):
                nc.tensor.matmul(wps, lhsT=ones8, rhs=ones8,
                                 start=(i == 0), stop=(i == n - 1),
                                 perf_mode=DR)

        warm(12)

        # ---- q' projection: q'^T[co, r] = A^T x^T + u (fp8 out, 16x scale) ----
        q_tiles = {}

        def make_qproj_steps(h, pools=None):
            qt = qpool.tile([P, ND, RQ], F8, tag="q8", name=f"q8_{h}")
            q_tiles[h] = qt

            def step(r2, co2, pool, tag):
                def go():
                    qps = pool.tile([P, CH], F32, tag=tag, name="qps")
                    nc.tensor.matmul(
                        qps,
                        lhsT=a8_sb[:, :, h, co2 * P:(co2 + 1) * P],
                        rhs=xbt_sb[:, :, r2 * CH:(r2 + 1) * CH],
                        start=True, stop=True, perf_mode=DR,
                    )
                    nc.vector.tensor_scalar_add(
                        out=qt[:, co2, r2 * CH:(r2 + 1) * CH], in0=qps,
                        scalar1=ub_sb[:, co2, h:h + 1],
                    )
                return go

            if pools is None:
                pools = [(ps_sm, "sm")] * 4
            return [step(r2, co2, *pools[r2 * ND + co2])
                    for r2 in range(NCH) for co2 in range(ND)]

        # h=0 projection up front: 4 MMs land in 4 different PSUM banks so
        # they stream back-to-back as soon as the a8/xbt DMA pieces arrive
        warm(28)
        for st in make_qproj_steps(0, pools=[(ps_sm, "sm"), (ps_rs, "rs"),
                                             (ps_ao, "ao"), (ps_ao, "ao")]):
            st()
        warm(12, pool=ps_sc, tag="sc")

        # ---- init acc = x + bfc_eff (residual folded in before head 0) ----
        def init_acc(i):
            nc.vector.scalar_tensor_tensor(
                out=acc_sb[:, i], in0=xr_sb[:, i], scalar=1.0, in1=bfc_sb,
                op0=OP.mult, op1=OP.add)

        # ---- LayerNorm: per-row stats, then a 4-row batched rsqrt chain ----
        out_r = out.rearrange("(n p) d -> p n d", p=P)
        ln_mv = {}

        def emit_ln_stats(i):
            stats = lnpool.tile([P, 6], F32, tag="stats")
            nc.vector.bn_stats(out=stats, in_=acc_sb[:, i])
            mv = lnpool.tile([P, 2], F32, tag="mv", name=f"mv{i}")
            nc.vector.bn_aggr(out=mv, in_=stats)
            ln_mv[i] = mv

        def emit_ln_finish(idxs):
            # rstd = 1/sqrt(var+eps) for all rows at once, DVE-only
            # (quake seed + 2 Newton steps) - no ACT table switch.
            n = len(idxs)
            ve = lnpool.tile([P, n], F32, tag="ve")
            for k, i in enumerate(idxs):
                nc.vector.tensor_scalar_add(out=ve[:, k:k + 1],
                                            in0=ln_mv[i][:, 1:2], scalar1=EPS)
            y = lnpool.tile([P, n], F32, tag="y")
            tn = lnpool.tile([P, n], F32, tag="tn")
            nc.vector.tensor_scalar(out=y.bitcast(I32), in0=ve.bitcast(I32),
                                    scalar1=1, scalar2=-1,
                                    op0=OP.arith_shift_right,
                                    op1=OP.bitwise_xor)
            nc.vector.tensor_scalar(out=y.bitcast(I32), in0=y.bitcast(I32),
                                    scalar1=0x5f3759df + 1, scalar2=None,
                                    op0=OP.add)
            # one Newton step: max rel err ~0.17%, well inside tolerance
            nc.vector.tensor_tensor(out=tn, in0=y, in1=y, op=OP.mult)
            nc.vector.tensor_tensor(out=tn, in0=tn, in1=ve, op=OP.mult)
            nc.vector.tensor_scalar(out=tn, in0=tn, scalar1=-0.5,
                                    scalar2=1.5, op0=OP.mult, op1=OP.add)
            nc.vector.tensor_tensor(out=y, in0=y, in1=tn, op=OP.mult)
            for k, i in enumerate(idxs):
                t = acc_sb[:, i]
                nc.vector.tensor_scalar(out=t, in0=t, scalar1=ln_mv[i][:, 0:1],
                                        scalar2=y[:, k:k + 1],
                                        op0=OP.subtract, op1=OP.mult)
            i0, i1 = min(idxs), max(idxs) + 1
            blk = acc_sb[:, i0:i1]
            nc.vector.tensor_tensor(out=blk, in0=blk, in1=gamma4_sb,
                                    op=OP.mult)
            nc.vector.tensor_tensor(out=blk, in0=blk, in1=beta4_sb, op=OP.add)
            nc.gpsimd.dma_start(out=out_r[:, i0:i1, :], in_=acc_sb[:, i0:i1])

        def emit_fc(ot_sb, fh, fch, final, r1s=None, last_part=True):
            # the final chunk's fc spreads over the freed rs/ao banks so the
            # 4 matmuls stream without waiting on per-bank DVE evacuation
            fin_pools = [(ps_sm, "sm"), (ps_rs, "rs"),
                         (ps_ao, "ao"), (ps_ao, "ao")]
            for r1 in (r1s if r1s is not None else range(CH // P)):
                idx = fch * (CH // P) + r1
                pool, ptag = fin_pools[r1] if (final and fch == NCH - 1) \
                    else (ps_sm, "sm")
                fcp = pool.tile([P, C], F32, tag=ptag, name="fcp")
                nc.tensor.matmul(
                    fcp,
                    lhsT=ot_sb[:, :, r1 * P:(r1 + 1) * P],
                    rhs=m8_sb[:, :, fh, :],
                    start=True, stop=True, perf_mode=DR,
                )
                # acc += fcp/2048 (ot carries 32x, M carries 64x)
                nc.vector.scalar_tensor_tensor(
                    out=acc_sb[:, idx], in0=fcp, scalar=1.0 / 2048.0,
                    in1=acc_sb[:, idx], op0=OP.mult, op1=OP.add)
                if final:
                    emit_ln_stats(idx)
            if final and last_part:
                emit_ln_finish([fch * (CH // P) + r1 for r1 in range(CH // P)])

        for i in range(NR):
            init_acc(i)
        fill_gb4()

        # ---- head loop, software-pipelined across chunk boundaries: each
        # chunk's last two rs/ao groups, softmax normalize, and fc are
        # deferred into the NEXT chunk's early iterations so neither the PE
        # nor the ACT ever drains at a boundary. ----
        def make_chunk_state(h, ch):
            qt = q_tiles[h]
            rsl = slice(ch * CH, (ch + 1) * CH)
            st = {
                "h": h, "ch": ch, "qt": qt, "rsl": rsl,
                "e8": epool.tile([P, NT, CH], F8, tag="e", name=f"e{h}{ch}"),
                "rs": None, "ao": None, "ot": None, "rcp": None,
            }
            return st

        def emit_rs(st, j):
            if st["rs"] is None:
                st["rs"] = ps_rs.tile([P, CH], F32, tag="rs", name="rs")
            nc.tensor.matmul(st["rs"], lhsT=ones8,
                             rhs=st["e8"][:, 2 * j:2 * j + 2, :],
                             start=(j == 0), stop=(j == NT // 2 - 1),
                             perf_mode=DR)
            if j == NT // 2 - 1:
                rcp = otpool.tile([P, CH], F32, tag="rcp")
                nc.vector.reciprocal_approx_fast(out=rcp, in_=st["rs"])
                st["rcp"] = rcp

        def emit_ao(st, j):
            if st["ao"] is None:
                st["ao"] = [ps_ao.tile([P, CH], F32, tag="ao", name=f"ao{c2}")
                            for c2 in range(ND)]
            for c2 in range(ND):
                nc.tensor.matmul(
                    st["ao"][c2],
                    lhsT=xb8_sb[:, 2 * j:2 * j + 2, c2 * P:(c2 + 1) * P],
                    rhs=st["e8"][:, 2 * j:2 * j + 2, :],
                    start=(j == 0), stop=(j == NT // 2 - 1),
                    perf_mode=DR,
                )
            if j == NT // 2 - 1:
                ot_sb = otpool.tile([P, ND, CH], F8, tag="ot")
                for c2 in range(ND):
                    nc.vector.tensor_tensor(out=ot_sb[:, c2], in0=st["ao"][c2],
                                            in1=st["rcp"][:], op=OP.mult)
                st["ot"] = ot_sb

        def emit_norm(st):
            rcp = otpool.tile([P, CH], F32, tag="rcp")
            nc.vector.reciprocal_approx_fast(out=rcp, in_=st["rs"])
            ot_sb = otpool.tile([P, ND, CH], F8, tag="ot")
            for c2 in range(ND):
                nc.vector.tensor_tensor(
                    out=ot_sb[:, c2], in0=st["ao"][c2], in1=rcp[:], op=OP.mult)
            st["ot"] = ot_sb

        # Uniform per-j schedule: every iteration carries sc-pair + one rs
        # group (3-deep lag) + one ao group (4-deep lag), so PE load per j
        # is nearly constant and the ACT exp stream never starves.
        chunks = [(h, ch) for h in range(H) for ch in range(NCH)]
        prev = None
        qsteps = []
        NH = NT // 2
        for h, ch in chunks:
            if ch == 0:
                qsteps = make_qproj_steps(h + 1) if h + 1 < H else []
            cur = make_chunk_state(h, ch)
            for j in range(NH):
                scp = ps_sc.tile([P, 2, CH], F32, tag="sc", name="scp")
                for tt in range(2):
                    t = 2 * j + tt
                    nc.tensor.matmul(
                        scp[:, tt],
                        lhsT=xbt_sb[:, :, t * P:(t + 1) * P],
                        rhs=cur["qt"][:, :, cur["rsl"]],
                        start=True, stop=True, perf_mode=DR,
                    )
                # e = exp(scores*SCALE - ln16), fp8; FD=1024 per op
                nc.scalar.activation(out=cur["e8"][:, 2 * j:2 * j + 2],
                                     in_=scp, func=AF.Exp, bias=expb,
                                     scale=ESCALE)
                if prev is not None:
                    if j == 0:
                        emit_rs(prev, NH - 3)
                        emit_ao(prev, NH - 3)
                        emit_rs(prev, NH - 2)
                        emit_ao(prev, NH - 2)
                    elif j == 1:
                        emit_rs(prev, NH - 1)
                        emit_ao(prev, NH - 1)
                if j >= 3:
                    emit_rs(cur, j - 3)
                    emit_ao(cur, j - 3)
                if j == 4 and prev is not None:
                    emit_fc(prev["ot"], prev["h"], prev["ch"],
                            prev["h"] == H - 1)
                    prev = None
                if j in (2, 5, 6, 7) and ch == 1 and qsteps:
                    qsteps.pop(0)()
            prev = cur
        # flush the final chunk
        for j in range(NH - 3, NH):
            emit_rs(prev, j)
            emit_ao(prev, j)
        emit_fc(prev["ot"], prev["h"], prev["ch"], True)

    nc.finalize()
    return nc


_NC = None


def _get_nc():
    global _NC
    if _NC is None:
        _NC = build_nc()
    return _NC


def make_in_maps(inputs):
    import ml_dtypes
    f8 = ml_dtypes.float8_e4m3

    x = np.asarray(inputs["x"], dtype=np.float32)
    Wq = np.asarray(inputs["Wq"], np.float32)
    Wk = np.asarray(inputs["Wk"], np.float32)
    Wv = np.asarray(inputs["Wv"], np.float32)
    Wfc = np.asarray(inputs["Wfc"], np.float32)
    bq = np.asarray(inputs["bq"], np.float32)
    bv = np.asarray(inputs["bv"], np.float32)
    bfc = np.asarray(inputs["bfc"], np.float32)
    gamma = np.asarray(inputs["gamma"], np.float32)
    beta = np.asarray(inputs["beta"], np.float32)

    # host-side folds (fp32)
    A = Wq @ Wk.transpose(0, 2, 1)                   # [H, C, C]
    u = np.einsum('hcd,hd->hc', Wk, bq)              # [H, C]
    M = Wv @ Wfc.reshape(H, C, C)                    # [H, C, C]
    bfc_eff = bfc + bv.ravel() @ Wfc

    a8_np = np.clip(16.0 * A, -240, 240).astype(f8)
    # [H, C, C] -> [P, ND, H, C]: (p, j, h, co) = A[h, j*128+p, co]
    a8_np = np.ascontiguousarray(
        a8_np.reshape(H, ND, P, C).transpose(2, 1, 0, 3))
    m8_np = np.clip(64.0 * M, -240, 240).astype(f8)
    m8_np = np.ascontiguousarray(
        m8_np.reshape(H, ND, P, C).transpose(2, 1, 0, 3))
    ub_np = np.ascontiguousarray((16.0 * u).reshape(H, ND, P).transpose(2, 1, 0))
    brow_np = np.ascontiguousarray(
        np.concatenate([bfc_eff.ravel(), gamma.ravel(), beta.ravel()]))

    shared = {"a8": a8_np, "m8": m8_np, "ub": ub_np, "brow": brow_np}
    in_maps = []
    for core in range(8):
        b, r0 = core // 2, (core % 2) * RQ
        x8r = np.roll(x[b].astype(f8), -r0, axis=0)          # [S, C] fp8
        m = dict(shared)
        # x^T: (p, j, t) = x8r[t, j*128+p]
        m["xbt8"] = np.ascontiguousarray(
            x8r.T.reshape(ND, P, S).transpose(1, 0, 2))
        # x rows: (p, n, c) = x8r[n*128+p, c]
        m["xb8"] = np.ascontiguousarray(
            x8r.reshape(NT, P, C).transpose(1, 0, 2))
        m["xqf"] = np.ascontiguousarray(
            x[b, r0:r0 + RQ].reshape(NR, P, C).transpose(1, 0, 2))
        in_maps.append(m)
    return in_maps


def assemble(results):
    out = np.empty((B, S, C), dtype=np.float32)
    for core in range(8):
        b, r0 = core // 2, (core % 2) * RQ
        out[b, r0:r0 + RQ] = results[core]["out"]
    return out


def kernel(**inputs) -> np.ndarray:
    from concourse.bass_utils import run_bass_kernel_spmd

    nc = _get_nc()
    in_maps = make_in_maps(inputs)
    res = run_bass_kernel_spmd(nc, in_maps, core_ids=list(range(8)))
    return assemble(res.results)


# revision 56
# speedup vs baseline: 1.0244x; 1.0054x over previous
"""Trainium2 Bass kernel for nn_MultiHeadAttention (B=4, S=2048, C=256, H=8).

Sharding: data-parallel over (batch, seq) - 8 cores, core i handles
batch b = i//2 and query rows r0 = (i%2)*1024 .. r0+1024.  No collectives;
host concatenates the 8 row-shards.

Algebraic folding (host side, fp32):
  scores = (x Wq + bq)(x Wk + bk)^T -> x A x^T + u.x_t  with A = Wq Wk^T,
  u = Wk bq (the bk term is constant per query row, softmax-invariant).
  attn (x Wv + bv) Wfc = (attn x) M + bv Wfc  with M = Wv Wfc.
  So the device only computes: q' = x A + u (one proj per head), scores
  against x^T directly, attn-times-x, then fc with M.  K and V projections
  and their SBUF copies are gone.

Precision: fp8e4 (DoubleRow, 2x contraction per pass) for q' proj, scores,
rowsum and attn*x; bf16 for the small fc; fp32 PSUM accumulation, softmax
normalization and LayerNorm in fp32.  A is scaled by 16 on host so fp8
quantization of q' (sigma~16) stays in the normal range; the activation
scale folds the 1/16 back.  exp is shifted by -ln(16) (softmax-invariant)
so e values stay well under the fp8e4 max of 240.

LayerNorm rstd = exp(-0.5*ln(var+eps)) keeps the whole kernel on one
activation table set (natural_log_exp_and_others) - no table switches.
"""

import sys

for _p in ("/opt/trn_rl_repo",):
    if _p not in sys.path:
        sys.path.insert(0, _p)

from contextlib import ExitStack

import numpy as np

import concourse.bass as bass
from concourse import bacc
import concourse.tile as tile
from concourse import mybir

P = 128
B, S, C, H = 4, 2048, 256, 8
RQ = 1024            # query rows per core
CH = 512             # query-row chunk (matmul N)
NCH = RQ // CH       # chunks per core = 2
NT = S // P          # key tiles = 16
ND = C // P          # feature tiles = 2
NR = RQ // P         # row tiles per core = 8
EPS = 1e-5
SCALE = 1.0 / np.sqrt(C)          # 1/16
ESCALE = float(SCALE / 16.0)      # activation scale: q' carries an extra 16x
LN16 = float(np.log(16.0))

F32 = mybir.dt.float32
I32 = mybir.dt.int32
BF16 = mybir.dt.bfloat16
F8 = mybir.dt.float8e4
F8E5 = mybir.dt.float8e5
AF = mybir.ActivationFunctionType
OP = mybir.AluOpType
DR = mybir.MatmulPerfMode.DoubleRow


def build_nc() -> bass.Bass:
    nc = bacc.Bacc(None)

    xbt8 = nc.declare_dram_parameter("xbt8", [P, ND, S], F8, isOutput=False)
    xb8 = nc.declare_dram_parameter("xb8", [P, NT, C], F8, isOutput=False)
    xqf = nc.declare_dram_parameter("xqf", [P, NR, C], F32, isOutput=False)
    a8 = nc.declare_dram_parameter("a8", [P, ND, H, C], F8, isOutput=False)
    m8 = nc.declare_dram_parameter("m8", [P, ND, H, C], F8, isOutput=False)
    ub = nc.declare_dram_parameter("ub", [P, ND, H], F32, isOutput=False)
    # brow = concat(bfc_eff [256], gamma [256], beta [256])
    brow = nc.declare_dram_parameter("brow", [3 * C], F32, isOutput=False)
    out = nc.declare_dram_parameter("out", [RQ, C], F32, isOutput=True)

    with tile.TileContext(nc) as tc, ExitStack() as ctx:
        singles = ctx.enter_context(tc.tile_pool(name="singles", bufs=1))
        qpool = ctx.enter_context(tc.tile_pool(name="qpool", bufs=2))
        epool = ctx.enter_context(tc.tile_pool(name="epool", bufs=2))
        otpool = ctx.enter_context(tc.tile_pool(name="otpool", bufs=2))
        lnpool = ctx.enter_context(tc.tile_pool(name="lnpool", bufs=4))

        ps_sc = ctx.enter_context(tc.tile_pool(name="ps_sc", bufs=2, space="PSUM"))
        ps_rs = ctx.enter_context(tc.tile_pool(name="ps_rs", bufs=1, space="PSUM"))
        ps_ao = ctx.enter_context(tc.tile_pool(name="ps_ao", bufs=2, space="PSUM"))
        ps_sm = ctx.enter_context(tc.tile_pool(name="ps_sm", bufs=1, space="PSUM"))

        # ---- constants ----
        # rowsum weights 1/32 so ot = 32*ao/rowsum stays in fp8e4 range
        # (|attn-weighted x| <= ~6, 32*6 = 192 < 240); fc de-scales by 1/2048.
        ones8 = singles.tile([P, ND, P], F8)
        nc.vector.memset(ones8, 1.0 / 32.0)
        expb = singles.tile([P, 1], F32)
        nc.vector.memset(expb, -LN16)
        # preload the exp table set while input DMAs are in flight
        tl_t = singles.tile([P, 1], F32)
        nc.scalar.activation(out=tl_t, in_=expb, func=AF.Exp, scale=1.0)

        # ---- input DMAs (all into persistent tiles).  The scalar (ACT)
        # queue gets ONLY the first a8 piece: DMA instructions on that queue
        # would otherwise delay the first exp by several us. ----
        ub_sb = singles.tile([P, ND, H], F32, tag="ub", name="ub_sb")
        nc.sync.dma_start(out=ub_sb, in_=ub[:])
        xbt_sb = singles.tile([P, ND, S], F8, tag="xbt", name="xbt_sb")
        for q4 in range(4):
            eng = nc.gpsimd if q4 % 2 == 0 else nc.sync
            eng.dma_start(out=xbt_sb[:, :, q4 * CH:(q4 + 1) * CH],
                          in_=xbt8[:, :, q4 * CH:(q4 + 1) * CH])
        a8_sb = singles.tile([P, ND, H, C], F8, tag="a8", name="a8_sb")
        nc.scalar.dma_start(out=a8_sb[:, :, 0:2], in_=a8[:, :, 0:2])
        xb8_sb = singles.tile([P, NT, C], F8, tag="xb8", name="xb8_sb")
        for q8_ in range(0, NT, 8):
            eng = nc.gpsimd if q8_ == 0 else nc.sync
            eng.dma_start(out=xb8_sb[:, q8_:q8_ + 8],
                          in_=xb8[:, q8_:q8_ + 8])
        for hh in range(2, H, 2):
            eng = nc.gpsimd if hh < 6 else nc.sync
            eng.dma_start(out=a8_sb[:, :, hh:hh + 2],
                          in_=a8[:, :, hh:hh + 2])
        m8_sb = singles.tile([P, ND, H, C], F8, tag="m8", name="m8_sb")
        for hh in range(0, H, 4):
            nc.sync.dma_start(out=m8_sb[:, :, hh:hh + 4],
                              in_=m8[:, :, hh:hh + 4])
        brow_sb = singles.tile([P, 3 * C], F32, tag="brow", name="brow_sb")
        brow_ap = brow[:]
        brow_bc = bass.AP(tensor=brow_ap.tensor, offset=brow_ap.offset,
                          ap=[[0, P]] + list(brow_ap.ap))
        nc.gpsimd.dma_start(out=brow_sb, in_=brow_bc)
        bfc_sb = brow_sb[:, 0:C]
        # gamma/beta replicated 4x so the LN epilogue applies them in one
        # FD=1024 op over 4 row-tiles
        gb4_sb = singles.tile([P, 2, 4, C], F32, tag="gb4", name="gb4_sb")
        gamma4_sb = gb4_sb[:, 0]
        beta4_sb = gb4_sb[:, 1]

        def fill_gb4():
            # deferred: DVE-FIFO ordering — must not precede the qproj casts
            for gi in range(2):
                for rep in range(4):
                    nc.vector.tensor_copy(
                        out=gb4_sb[:, gi, rep],
                        in_=brow_sb[:, (1 + gi) * C:(2 + gi) * C])
        xr_sb = singles.tile([P, NR, C], F32, tag="xr", name="xr_sb")
        for q8_ in range(0, NR, 4):
            eng = nc.gpsimd if q8_ == 0 else nc.sync
            eng.dma_start(out=xr_sb[:, q8_:q8_ + 4],
                          in_=xqf[:, q8_:q8_ + 4])

        # ---- fc accumulator (fp32, SBUF) ----
        acc_sb = singles.tile([P, NR, C], F32, tag="acc", name="acc_sb")

        # ---- warmup: get the HAM clock gate to 2.4 GHz while DMAs land.
        # One PSUM accumulation group -> back-to-back MMs, no inter-MM sems.
        def warm(n, pool=None, tag=None):
            wps = (pool or ps_rs).tile([P, P], F32, tag=tag or "rs",
                                       name="wps")
            for i in range(n):
                nc.tensor.matmul(wps, lhsT=ones8, rhs=ones8,
                                 start=(i == 0), stop=(i == n - 1),
                                 perf_mode=DR)

        warm(12)

        # ---- q' projection: q'^T[co, r] = A^T x^T + u (fp8 out, 16x scale) ----
        q_tiles = {}

        def make_qproj_steps(h, pools=None):
            qt = qpool.tile([P, ND, RQ], F8, tag="q8", name=f"q8_{h}")
            q_tiles[h] = qt

            def step(r2, co2, pool, tag):
                def go():
                    qps = pool.tile([P, CH], F32, tag=tag, name="qps")
                    nc.tensor.matmul(
                        qps,
                        lhsT=a8_sb[:, :, h, co2 * P:(co2 + 1) * P],
                        rhs=xbt_sb[:, :, r2 * CH:(r2 + 1) * CH],
                        start=True, stop=True, perf_mode=DR,
                    )
                    nc.vector.tensor_scalar_add(
                        out=qt[:, co2, r2 * CH:(r2 + 1) * CH], in0=qps,
                        scalar1=ub_sb[:, co2, h:h + 1],
                    )
                return go

            if pools is None:
                pools = [(ps_sm, "sm")] * 4
            return [step(r2, co2, *pools[r2 * ND + co2])
                    for r2 in range(NCH) for co2 in range(ND)]

        # h=0 projection up front: 4 MMs land in 4 different PSUM banks so
        # they stream back-to-back as soon as the a8/xbt DMA pieces arrive
        warm(28)
        for st in make_qproj_steps(0, pools=[(ps_sm, "sm"), (ps_rs, "rs"),
                                             (ps_ao, "ao"), (ps_ao, "ao")]):
            st()
        warm(12, pool=ps_sc, tag="sc")

        # ---- init acc = x + bfc_eff (residual folded in before head 0) ----
        def init_acc(i):
            nc.vector.scalar_tensor_tensor(
                out=acc_sb[:, i], in0=xr_sb[:, i], scalar=1.0, in1=bfc_sb,
                op0=OP.mult, op1=OP.add)

        # ---- LayerNorm: per-row stats, then a 4-row batched rsqrt chain ----
        out_r = out.rearrange("(n p) d -> p n d", p=P)
        ln_mv = {}

        def emit_ln_stats(i):
            stats = lnpool.tile([P, 6], F32, tag="stats")
            nc.vector.bn_stats(out=stats, in_=acc_sb[:, i])
            mv = lnpool.tile([P, 2], F32, tag="mv", name=f"mv{i}")
            nc.vector.bn_aggr(out=mv, in_=stats)
            ln_mv[i] = mv

        def emit_ln_finish(idxs):
            # rstd = 1/sqrt(var+eps) for all rows at once, DVE-only
            # (quake seed + 2 Newton steps) - no ACT table switch.
            n = len(idxs)
            ve = lnpool.tile([P, n], F32, tag="ve")
            for k, i in enumerate(idxs):
                nc.vector.tensor_scalar_add(out=ve[:, k:k + 1],
                                            in0=ln_mv[i][:, 1:2], scalar1=EPS)
            y = lnpool.tile([P, n], F32, tag="y")
            tn = lnpool.tile([P, n], F32, tag="tn")
            nc.vector.tensor_scalar(out=y.bitcast(I32), in0=ve.bitcast(I32),
                                    scalar1=1, scalar2=-1,
                                    op0=OP.arith_shift_right,
                                    op1=OP.bitwise_xor)
            nc.vector.tensor_scalar(out=y.bitcast(I32), in0=y.bitcast(I32),
                                    scalar1=0x5f3759df + 1, scalar2=None,
                                    op0=OP.add)
            # one Newton step: max rel err ~0.17%, well inside tolerance
            nc.vector.tensor_tensor(out=tn, in0=y, in1=y, op=OP.mult)
            nc.vector.tensor_tensor(out=tn, in0=tn, in1=ve, op=OP.mult)
            nc.vector.tensor_scalar(out=tn, in0=tn, scalar1=-0.5,
                                    scalar2=1.5, op0=OP.mult, op1=OP.add)
            nc.vector.tensor_tensor(out=y, in0=y, in1=tn, op=OP.mult)
            for k, i in enumerate(idxs):
                t = acc_sb[:, i]
                nc.vector.tensor_scalar(out=t, in0=t, scalar1=ln_mv[i][:, 0:1],
                                        scalar2=y[:, k:k + 1],
                                        op0=OP.subtract, op1=OP.mult)
            i0, i1 = min(idxs), max(idxs) + 1
            blk = acc_sb[:, i0:i1]
            nc.vector.tensor_tensor(out=blk, in0=blk, in1=gamma4_sb,
                                    op=OP.mult)
            nc.vector.tensor_tensor(out=blk, in0=blk, in1=beta4_sb, op=OP.add)
            nc.gpsimd.dma_start(out=out_r[:, i0:i1, :], in_=acc_sb[:, i0:i1])

        def emit_fc(ot_sb, fh, fch, final, r1s=None, last_part=True):
            # the final chunk's fc spreads over the freed rs/ao banks so the
            # 4 matmuls stream without waiting on per-bank DVE evacuation
            fin_pools = [(ps_sm, "sm"), (ps_rs, "rs"),
                         (ps_ao, "ao"), (ps_ao, "ao")]
            for r1 in (r1s if r1s is not None else range(CH // P)):
                idx = fch * (CH // P) + r1
                pool, ptag = fin_pools[r1] if (final and fch == NCH - 1) \
                    else (ps_sm, "sm")
                fcp = pool.tile([P, C], F32, tag=ptag, name="fcp")
                nc.tensor.matmul(
                    fcp,
                    lhsT=ot_sb[:, :, r1 * P:(r1 + 1) * P],
                    rhs=m8_sb[:, :, fh, :],
                    start=True, stop=True, perf_mode=DR,
                )
                # acc += fcp/2048 (ot carries 32x, M carries 64x)
                nc.vector.scalar_tensor_tensor(
                    out=acc_sb[:, idx], in0=fcp, scalar=1.0 / 2048.0,
                    in1=acc_sb[:, idx], op0=OP.mult, op1=OP.add)
                if final:
                    emit_ln_stats(idx)
            if final and last_part:
                emit_ln_finish([fch * (CH // P) + r1 for r1 in range(CH // P)])

        for i in range(NR):
            init_acc(i)
        fill_gb4()

        # ---- head loop, software-pipelined across chunk boundaries: each
        # chunk's last two rs/ao groups, softmax normalize, and fc are
        # deferred into the NEXT chunk's early iterations so neither the PE
        # nor the ACT ever drains at a boundary. ----
        def make_chunk_state(h, ch):
            qt = q_tiles[h]
            rsl = slice(ch * CH, (ch + 1) * CH)
            st = {
                "h": h, "ch": ch, "qt": qt, "rsl": rsl,
                "e8": epool.tile([P, NT, CH], F8E5, tag="e", name=f"e{h}{ch}"),
                "rs": None, "ao": None, "ot": None, "rcp": None,
            }
            return st

        def emit_rs(st, j):
            if st["rs"] is None:
                st["rs"] = ps_rs.tile([P, CH], F32, tag="rs", name="rs")
            nc.tensor.matmul(st["rs"], lhsT=ones8,
                             rhs=st["e8"][:, 2 * j:2 * j + 2, :],
                             start=(j == 0), stop=(j == NT // 2 - 1),
                             perf_mode=DR)
            if j == NT // 2 - 1:
                rcp = otpool.tile([P, CH], F32, tag="rcp")
                nc.vector.reciprocal_approx_fast(out=rcp, in_=st["rs"])
                st["rcp"] = rcp

        def emit_ao(st, j):
            if st["ao"] is None:
                st["ao"] = [ps_ao.tile([P, CH], F32, tag="ao", name=f"ao{c2}")
                            for c2 in range(ND)]
            for c2 in range(ND):
                nc.tensor.matmul(
                    st["ao"][c2],
                    lhsT=xb8_sb[:, 2 * j:2 * j + 2, c2 * P:(c2 + 1) * P],
                    rhs=st["e8"][:, 2 * j:2 * j + 2, :],
                    start=(j == 0), stop=(j == NT // 2 - 1),
                    perf_mode=DR,
                )
            if j == NT // 2 - 1:
                ot_sb = otpool.tile([P, ND, CH], F8, tag="ot")
                for c2 in range(ND):
                    nc.vector.tensor_tensor(out=ot_sb[:, c2], in0=st["ao"][c2],
                                            in1=st["rcp"][:], op=OP.mult)
                st["ot"] = ot_sb

        def emit_norm(st):
            rcp = otpool.tile([P, CH], F32, tag="rcp")
            nc.vector.reciprocal_approx_fast(out=rcp, in_=st["rs"])
            ot_sb = otpool.tile([P, ND, CH], F8, tag="ot")
            for c2 in range(ND):
                nc.vector.tensor_tensor(
                    out=ot_sb[:, c2], in0=st["ao"][c2], in1=rcp[:], op=OP.mult)
            st["ot"] = ot_sb

        # Uniform per-j schedule: every iteration carries sc-pair + one rs
        # group (3-deep lag) + one ao group (4-deep lag), so PE load per j
        # is nearly constant and the ACT exp stream never starves.
        chunks = [(h, ch) for h in range(H) for ch in range(NCH)]
        prev = None
        qsteps = []
        NH = NT // 2
        for h, ch in chunks:
            if ch == 0:
                qsteps = make_qproj_steps(h + 1) if h + 1 < H else []
            cur = make_chunk_state(h, ch)
            for j in range(NH):
                scp = ps_sc.tile([P, 2, CH], F32, tag="sc", name="scp")
                for tt in range(2):
                    t = 2 * j + tt
                    nc.tensor.matmul(
                        scp[:, tt],
                        lhsT=xbt_sb[:, :, t * P:(t + 1) * P],
                        rhs=cur["qt"][:, :, cur["rsl"]],
                        start=True, stop=True, perf_mode=DR,
                    )
                # e = exp(scores*SCALE) in fp8e5 (max 57344, no range
                # shift needed); immediate zero bias skips the bias-AP read
                nc.scalar.activation(out=cur["e8"][:, 2 * j:2 * j + 2],
                                     in_=scp, func=AF.Exp, scale=ESCALE)
                if prev is not None:
                    if j == 0:
                        emit_rs(prev, NH - 3)
                        emit_ao(prev, NH - 3)
                        emit_rs(prev, NH - 2)
                        emit_ao(prev, NH - 2)
                    elif j == 1:
                        emit_rs(prev, NH - 1)
                        emit_ao(prev, NH - 1)
                if j >= 3:
                    emit_rs(cur, j - 3)
                    emit_ao(cur, j - 3)
                if j == 4 and prev is not None:
                    emit_fc(prev["ot"], prev["h"], prev["ch"],
                            prev["h"] == H - 1)
                    prev = None
                if j in (2, 5, 6, 7) and ch == 1 and qsteps:
                    qsteps.pop(0)()
            prev = cur
        # flush the final chunk
        for j in range(NH - 3, NH):
            emit_rs(prev, j)
            emit_ao(prev, j)
        emit_fc(prev["ot"], prev["h"], prev["ch"], True)

    nc.finalize()
    return nc


_NC = None


def _get_nc():
    global _NC
    if _NC is None:
        _NC = build_nc()
    return _NC


def make_in_maps(inputs):
    import ml_dtypes
    f8 = ml_dtypes.float8_e4m3

    x = np.asarray(inputs["x"], dtype=np.float32)
    Wq = np.asarray(inputs["Wq"], np.float32)
    Wk = np.asarray(inputs["Wk"], np.float32)
    Wv = np.asarray(inputs["Wv"], np.float32)
    Wfc = np.asarray(inputs["Wfc"], np.float32)
    bq = np.asarray(inputs["bq"], np.float32)
    bv = np.asarray(inputs["bv"], np.float32)
    bfc = np.asarray(inputs["bfc"], np.float32)
    gamma = np.asarray(inputs["gamma"], np.float32)
    beta = np.asarray(inputs["beta"], np.float32)

    # host-side folds (fp32)
    A = Wq @ Wk.transpose(0, 2, 1)                   # [H, C, C]
    u = np.einsum('hcd,hd->hc', Wk, bq)              # [H, C]
    M = Wv @ Wfc.reshape(H, C, C)                    # [H, C, C]
    bfc_eff = bfc + bv.ravel() @ Wfc

    a8_np = np.clip(16.0 * A, -240, 240).astype(f8)
    # [H, C, C] -> [P, ND, H, C]: (p, j, h, co) = A[h, j*128+p, co]
    a8_np = np.ascontiguousarray(
        a8_np.reshape(H, ND, P, C).transpose(2, 1, 0, 3))
    m8_np = np.clip(64.0 * M, -240, 240).astype(f8)
    m8_np = np.ascontiguousarray(
        m8_np.reshape(H, ND, P, C).transpose(2, 1, 0, 3))
    ub_np = np.ascontiguousarray((16.0 * u).reshape(H, ND, P).transpose(2, 1, 0))
    brow_np = np.ascontiguousarray(
        np.concatenate([bfc_eff.ravel(), gamma.ravel(), beta.ravel()]))

    shared = {"a8": a8_np, "m8": m8_np, "ub": ub_np, "brow": brow_np}
    in_maps = []
    for core in range(8):
        b, r0 = core // 2, (core % 2) * RQ
        x8r = np.roll(x[b].astype(f8), -r0, axis=0)          # [S, C] fp8
        m = dict(shared)
        # x^T: (p, j, t) = x8r[t, j*128+p]
        m["xbt8"] = np.ascontiguousarray(
            x8r.T.reshape(ND, P, S).transpose(1, 0, 2))
        # x rows: (p, n, c) = x8r[n*128+p, c]
        m["xb8"] = np.ascontiguousarray(
            x8r.reshape(NT, P, C).transpose(1, 0, 2))
        m["xqf"] = np.ascontiguousarray(
            x[b, r0:r0 + RQ].reshape(NR, P, C).transpose(1, 0, 2))
        in_maps.append(m)
    return in_maps


def assemble(results):
    out = np.empty((B, S, C), dtype=np.float32)
    for core in range(8):
        b, r0 = core // 2, (core % 2) * RQ
        out[b, r0:r0 + RQ] = results[core]["out"]
    return out


def kernel(**inputs) -> np.ndarray:
    from concourse.bass_utils import run_bass_kernel_spmd

    nc = _get_nc()
    in_maps = make_in_maps(inputs)
    res = run_bass_kernel_spmd(nc, in_maps, core_ids=list(range(8)))
    return assemble(res.results)
